# revision 37
# baseline (speedup 1.0000x reference)
"""COOTensorProduct kernel for 8 Trainium2 NeuronCores (Tile, bf16).

Math: out[b, h] = sum_{i,j} cb[h, i*64+j] * in1[b, i] * in2[b, j]
with in1/in2 [4096, 64], cb [4096, 4096] (Clebsch-Gordan coupling for
irreps '4x0e+4x1o+4x2e+4x3o' x same -> all l3).

cb is block-structured: the 16 (l1,l2) pair couplings are square
matrices that pack block-diagonally into two 128x128 stationaries.
Per core (512 batch rows), for each group s and multiplicity pair
(u, v): rhs = g1[s,u] * g2[s,v] elementwise, psum = W_s.T @ rhs.

Device pipeline is bf16 end-to-end (rel-err budget 2e-2; this
measures 3.66e-3, bit-deterministic): single-pass matmuls (fp32
needs a LOW/HIGH dual pass), half the DMA bytes, 2x DVE rate.

Structure per core: 3 packed input DMAs ([g2_0|g1_0] split so the
first product gates on only the slice it reads, [w0|w1] on the sync
ring, [g2_1|g1_1] FIFO'd behind ring A), an ACT spline-table warm-up
off the first drain's critical path, then 8 supertiles: one DVE
product [128,2048] (in0 broadcast over v), 4 matmuls into a 4-bank
PSUM tile, a PSUM drain (3 of every 4 on scalar ACTIVATE, u=3 on
vector; the last supertile drains in parallel halves and ships on
two DMA rings to shorten the tail), and an output DMA. Buffer
rings: rhs x4, psum x2, ot x4.

Uses the Tile framework deliberately: a hand-scheduled raw-bass
variant of this kernel (kernel_ship.py) is ~15% faster but its
drain timing races the engines' completion-event-vs-write-back gap
and corrupts intermittently under thermal throttle; Tile's
conservative semaphore placement has produced bit-identical output
on every run. Hardware constraints that shaped this: DMA
per-partition chunk sizes must be multiples of 512B; HWDGE DMA
rings pay a ~0.7us completion receipt per transfer (FIFO), so
fewer/packed input DMAs win; drain capacity (PSUM reads are
port-bound at 4B/lane/cyc on scalar and vector) is the steady-state
binder, not the product chain.
"""

import json
import numpy as np
import ml_dtypes

BF16 = ml_dtypes.bfloat16

# ---------------------------------------------------------------- problem
B = 4096
DIM = 64
NCORES = 8
BPC = B // NCORES  # 512 batch rows per core
LMAX = 3
NMULT = 4  # multiplicity of each l in '4x0e+4x1o+4x2e+4x3o'
LS = [l for l in range(LMAX + 1) for _ in range(NMULT)]

# block-diagonal packing of the 16 (l1,l2) pair matrices into 2 stationaries
PAIRS_A = [(3, 3), (3, 2), (2, 3), (1, 1)]
PAIRS_B = [(2, 2), (1, 3), (3, 1), (1, 2), (2, 1), (0, 3), (3, 0),
           (0, 2), (2, 0), (0, 1), (1, 0), (0, 0)]

_decomp_cache = None
_nc_cache = None
WARMUP_LDW = 0  # dummy PE Ldweights injected into the preamble (0 = off)
DEDUP_LDW = 1  # drop consecutive identical PE Ldweights in BIR
PE_FILL_LDW = 4  # filler Ldweights after each supertile (0 = off)
MAX_SEM_NUM = 0  # if >0, pass --max-sem-num to walrus (shrinks the fixed
                 # full-semaphore-file reset sweep in the NEFF epilogue)
CACHE_TAG = "v5ldw"  # bumped when compile flags change (busts the NEFF cache)


def _patch_walrus_flags():
    if not MAX_SEM_NUM:
        return
    import concourse.bass_utils as bu
    if getattr(bu, "_kernel_flag_patch", None) == MAX_SEM_NUM:
        return
    orig = getattr(bu, "_orig_get_walrus_args", bu.get_walrus_args)
    bu._orig_get_walrus_args = orig

    def patched(*a, **k):
        return orig(*a, **k) + [f"--max-sem-num={MAX_SEM_NUM}"]

    bu.get_walrus_args = patched
    bu._kernel_flag_patch = MAX_SEM_NUM


def _col_start(l, u):
    return sum((2 * ll + 1) * NMULT for ll in range(l)) + u * (2 * l + 1)


def _build_decomp():
    """Index bookkeeping only (no numerics): which cb entries form the two
    stationary matrices, which in1/in2 columns feed each partition row,
    and which output row h each psum row maps to."""
    global _decomp_cache
    if _decomp_cache is not None:
        return _decomp_cache

    # replicate build_cb_matrix's row layout
    layout = {}
    idx1 = 0
    for l1 in LS:
        idx2 = 0
        for l2 in LS:
            for l3 in range(abs(l1 - l2), l1 + l2 + 1):
                layout.setdefault(l3, []).append((l1, l2, idx1 * DIM + idx2))
            idx2 += 2 * l2 + 1
        idx1 += 2 * l1 + 1
    entry_row = {}
    row = 0
    for l3 in sorted(layout):
        for (l1, l2, co) in sorted(layout[l3], key=lambda x: x[0] * LMAX + x[1]):
            entry_row[(l3, co)] = row
            row += 2 * l3 + 1
    assert row == B

    groups = []
    for pairs in (PAIRS_A, PAIRS_B):
        assert sum((2 * a + 1) * (2 * b + 1) for a, b in pairs) == 128
        c1 = np.zeros((NMULT, 128), dtype=np.int64)
        c2 = np.zeros((NMULT, 128), dtype=np.int64)
        h_of = np.zeros((NMULT, NMULT, 128), dtype=np.int64)
        w_k, w_m, w_h, w_c = [], [], [], []  # W[k,m] = cb[h, c]
        off = 0
        for (l1, l2) in pairs:
            n1, n2 = 2 * l1 + 1, 2 * l2 + 1
            kp = n1 * n2
            kk = np.arange(kp)
            m1, m2 = kk // n2, kk % n2
            for u in range(NMULT):
                c1[u, off:off + kp] = _col_start(l1, u) + m1
            for v in range(NMULT):
                c2[v, off:off + kp] = _col_start(l2, v) + m2
            mm = 0
            for l3 in range(abs(l1 - l2), l1 + l2 + 1):
                n3 = 2 * l3 + 1
                h0 = entry_row[(l3, _col_start(l1, 0) * DIM + _col_start(l2, 0))]
                km, m3m = np.meshgrid(kk, np.arange(n3), indexing="ij")
                w_k.append((off + km).ravel())
                w_m.append((off + mm + m3m).ravel())
                w_h.append((h0 + m3m).ravel())
                w_c.append(((_col_start(l1, 0) + m1[km.ravel()]) * DIM
                            + (_col_start(l2, 0) + m2[km.ravel()])))
                for u in range(NMULT):
                    for v in range(NMULT):
                        h = entry_row[(l3, _col_start(l1, u) * DIM + _col_start(l2, v))]
                        h_of[u, v, off + mm:off + mm + n3] = np.arange(h, h + n3)
                mm += n3
            off += kp
        groups.append({
            "c1": c1, "c2": c2, "h_of": h_of,
            "w_k": np.concatenate(w_k), "w_m": np.concatenate(w_m),
            "w_h": np.concatenate(w_h), "w_c": np.concatenate(w_c),
        })

    # global output row -> h map: tile t = S*16 + u*4 + v holds rows
    # t*128 + mm  ->  h_of[S][u, v, mm]
    hglob = np.zeros(32 * 128, dtype=np.int64)
    for s, g in enumerate(groups):
        for u in range(NMULT):
            for v in range(NMULT):
                t = s * 16 + u * 4 + v
                hglob[t * 128:(t + 1) * 128] = g["h_of"][u, v]
    _decomp_cache = (groups, hglob)
    return _decomp_cache


def _split_waits(bir_bytes):
    """This container's walrus build rejects >1 sync-wait per instruction
    ("Too many sync wait commands"). Hoist extra waits onto standalone
    EventSemaphore instructions on the same engine (same lowering raw
    bass wait_ge uses)."""
    bir = json.loads(bir_bytes)
    n = 0
    for fn in bir["functions"]:
        for blk in fn["blocks"]:
            out = []
            for inst in blk["instructions"]:
                si = inst.get("sync_info")
                waits = (si or {}).get("on_wait") or []
                if len(waits) > 1:
                    for w in waits[:-1]:
                        n += 1
                        out.append({
                            "debug": inst.get("debug", 0),
                            "engine": inst["engine"],
                            "ins": [], "outs": [],
                            "name": f"I-wsplit-{n}",
                            "opcode": "EventSemaphore",
                            "sync_info": {"on_update": [], "on_wait": [w]},
                        })
                    si["on_wait"] = [waits[-1]]
                out.append(inst)
            blk["instructions"] = out
    return json.dumps(bir).encode()


def _dedup_ldweights(bir_bytes):
    """Drop PE Ldweights whose source AP is identical to the previous
    Ldweights on the engine (the stationary only changes between the two
    s-groups; Tile re-emits it for every matmul).  Saves ~107ns of PE per
    matmul — PE per-supertile drops from ~1.93us to ~1.6us, which matters
    because PE is co-pacer with the ACT drain chain.  Waits on a dropped
    Ldweights are re-homed onto a standalone EventSemaphore."""
    if not DEDUP_LDW:
        return bir_bytes
    bir = json.loads(bir_bytes)
    n = 0
    nfill = 0
    for fn in bir["functions"]:
        for blk in fn["blocks"]:
            out = []
            last_key = None
            last_ldw = None
            mm_count = 0
            for inst in blk["instructions"]:
                if inst.get("engine") != "PE":
                    out.append(inst)
                    continue
                if inst["opcode"] != "Ldweights":
                    if inst["opcode"] not in ("Matmult", "EventSemaphore"):
                        # unknown PE op: assume it clobbers the stationary
                        last_key = None
                    out.append(inst)
                    if (PE_FILL_LDW and inst["opcode"] == "Matmult"
                            and last_ldw is not None):
                        mm_count += 1
                        # after each supertile's 4 matmuls, re-load the
                        # current stationary a few times: harmless, and
                        # plugs the PE idle gap so the p-state ramp
                        # (reset by >100ns idle) survives between tiles
                        if mm_count % 4 == 0 and mm_count < 32:
                            for _ in range(PE_FILL_LDW):
                                nfill += 1
                                w = json.loads(json.dumps(last_ldw))
                                w["name"] = f"I-ldwfill-{nfill}"
                                w["sync_info"] = {"on_update": [],
                                                  "on_wait": []}
                                out.append(w)
                    continue
                last_ldw = inst
                key = json.dumps(inst.get("ins"), sort_keys=True)
                if key == last_key:
                    waits = (inst.get("sync_info") or {}).get("on_wait") or []
                    for w in waits:
                        n += 1
                        out.append({
                            "debug": inst.get("debug", 0), "engine": "PE",
                            "ins": [], "outs": [],
                            "name": f"I-ldwsplit-{n}",
                            "opcode": "EventSemaphore",
                            "sync_info": {"on_update": [], "on_wait": [w]},
                        })
                    continue
                last_key = key
                out.append(inst)
            blk["instructions"] = out
    return json.dumps(bir).encode()


def _hoist_preamble(bir_bytes):
    """BIR surgery that front-loads work into the main block, ahead of
    Tile's entry barrier (the fixed walrus preamble — start barrier +
    DGE-state TENSOR_LOAD — still runs first, so DGE is armed and inputs
    are staged in DRAM):

    1. input DMACopy issues (wait-free) move to main: transfers stream
       during the remaining ~2.5us of preamble.  Gating still works
       because the completion sems are zero-initialized at NEFF load.
    2. the scr warm-up Activation (which exists only to trigger the lazy
       ~1.3us ACT_TABLE_LOAD) moves to main with its wait stripped — it
       deliberately reads garbage, its output is never consumed.
    3. inject dummy PE Ldweights reading tC (garbage until inC lands —
       discarded: every real matmul is preceded by a real Ldweights).
       The PE p-state ramp needs ~3us of sustained activity to reach
       2.4 GHz; cold matmuls run at ~1.7x the warm cost, so ramping
       during the preamble buys back most of the first supertiles."""
    bir = json.loads(bir_bytes)
    for fn in bir["functions"]:
        blocks = fn["blocks"]
        main = next(b for b in blocks if b["name"] == "main")
        hoisted = []
        ldw_proto = None
        t1b_sem = None
        inc_dma = None
        for blk in blocks:
            if blk is main:
                continue
            keep = []
            for inst in blk["instructions"]:
                si = inst.get("sync_info") or {}
                in_refs = {a.get("memref", "") for a in inst.get("ins", [])}
                out_refs = {a.get("memref", "") for a in inst.get("outs", [])}
                if (inst["opcode"] == "DMACopy"
                        and not (si.get("on_wait") or [])
                        and any(n.startswith("inA") for n in in_refs)):
                    if len(hoisted) == 1:  # second inA transfer = T1b
                        t1b_sem = si["on_update"][0]
                    hoisted.append(inst)
                    continue
                if (inst["opcode"] == "DMACopy"
                        and any(n.startswith("inC") for n in in_refs)):
                    inc_dma = inst
                if (inst["opcode"] == "Activation"
                        and any(n.startswith("scr") for n in out_refs)):
                    # strip only the wait — its sem *updates* are counted
                    # by downstream gates and must keep firing
                    si["on_wait"] = []
                    hoisted.append(inst)
                    continue
                if inst["opcode"] == "Ldweights" and ldw_proto is None:
                    ldw_proto = inst
                keep.append(inst)
            blk["instructions"] = keep
        # gate inC's (scalar-ring) issue on T1b completion so its packets
        # can't steal DMA-engine slots from the transfers gating s=0
        if inc_dma is not None and t1b_sem is not None:
            inc_dma["sync_info"]["on_wait"] = [{
                "ant_name": t1b_sem["ant_name"], "id": t1b_sem["id"],
                "sync_type": "semaphore", "wait_mode": "sem-ge-imm",
                "wait_value": 16}]
        warmups = []
        if WARMUP_LDW and ldw_proto is not None:
            # clone a real (already matmul-weights-lowered) Ldweights
            # verbatim — same source AP, waits stripped.  It reads the
            # weights region of tA (garbage until T1a lands; discarded —
            # every real matmul is preceded by its own real Ldweights).
            for i in range(WARMUP_LDW):
                w = json.loads(json.dumps(ldw_proto))
                w["name"] = f"I-wuldw-{i}"
                w["sync_info"] = {"on_update": [], "on_wait": []}
                warmups.append(w)
        # no-op marker carrying the compile-config tag: changing the tag
        # changes the BIR bytes, busting the NEFF cache when flags change
        nonce = [{
            "debug": 0, "engine": "SP", "ins": [], "outs": [],
            "name": f"I-cfg-{CACHE_TAG}", "opcode": "EventSemaphore",
            "sync_info": {"on_update": [], "on_wait": []},
        }]
        # keep the Call pseudo-instruction first
        insts = main["instructions"]
        call_end = 1 if insts and insts[0]["opcode"] == "Call" else 0
        main["instructions"] = (insts[:call_end] + hoisted + warmups
                                + nonce + insts[call_end:])
    return json.dumps(bir).encode()



def _build_nc():
    """Tile-framework bf16 pipeline. Slower steady-state than the raw
    hand-scheduled version (~42 vs ~31us) but the Tile scheduler's
    conservative semaphore placement has never produced a corrupt run,
    unlike the raw version whose drain timing races the PE/DVE
    write-back under thermal throttle."""
    global _nc_cache
    if _nc_cache is not None:
        return _nc_cache
    import concourse.bass as bass
    import concourse.mybir as mybir
    from concourse.tile import TileContext

    bf16 = mybir.dt.bfloat16
    f32 = mybir.dt.float32
    nc = bass.Bass()
    # inA = [w0|w1 (256) | g1_0u0 (BPC) | g2_0 (4BPC) | g1_0u1..3 (3BPC)],
    # inC = [g2_1 | g1_1].  All four transfers ride the sync ring FIFO in
    # dependency order: T1a (weights + product-u0v0 operands) first at
    # full rate, then g2_0 v1..3, then g1_0 u1..3, then inC.  The HWDGE
    # ring pays a ~0.7us completion receipt per DMA, so chunks stay big.
    W0 = 256  # weights columns at the front of tA
    inA = nc.dram_tensor("inA", [128, W0 + 8 * BPC], bf16,
                         kind="ExternalInput")
    inC = nc.dram_tensor("inC", [128, 8 * BPC], bf16, kind="ExternalInput")
    o = nc.dram_tensor("o", [8, 128, 4 * BPC], bf16, kind="ExternalOutput")

    with TileContext(nc) as tc:
        with (
            tc.tile_pool(name="sb", bufs=1) as sb,
            tc.tile_pool(name="psum", bufs=2, space="PSUM") as psumpool,
        ):
            tA = sb.tile([128, W0 + 8 * BPC], bf16, tag="tA", name="tA",
                         bufs=1)
            c1 = W0 + 2 * BPC
            c2 = W0 + 5 * BPC
            nc.sync.dma_start(out=tA[:, :c1], in_=inA[:, :c1])
            nc.sync.dma_start(out=tA[:, c1:c2], in_=inA[:, c1:c2])
            nc.sync.dma_start(out=tA[:, c2:], in_=inA[:, c2:])
            # inC issues from scalar (own ring), artificially gated on the
            # T1b completion sem (added in _hoist_preamble): the 16 DMA
            # engines drain their ring shares independently, so an
            # ungated inC steals ~2us of bandwidth from the T1/T2
            # transfers that gate all of s=0's compute.
            tC = sb.tile([128, 8 * BPC], bf16, tag="tC", name="tC", bufs=1)
            nc.scalar.dma_start(out=tC, in_=inC[:, :])
            g2t = [tA[:, W0 + BPC:W0 + 5 * BPC], tC[:, :4 * BPC]]
            wt = [tA[:, :128], tA[:, 128:256]]

            def g1slice(s, u):
                if s == 1:
                    return tC[:, (4 + u) * BPC:(5 + u) * BPC]
                if u == 0:
                    return tA[:, W0:W0 + BPC]
                return tA[:, W0 + (4 + u) * BPC:W0 + (5 + u) * BPC]

            # warm the ACT COPY spline table (lazy ACT_TABLE_LOAD is
            # ~1.3us) off the first psum drain's critical path; hoisted
            # to the preamble (reads garbage, output never consumed)
            scr = sb.tile([128, 16], bf16, tag="scr", name="scr", bufs=1)
            nc.scalar.copy(out=scr, in_=tA[:, :16])

            for s in range(2):
                for u in range(NMULT):
                    rhs = sb.tile([128, 4 * BPC], bf16, tag="rhs", bufs=4)
                    in0 = g1slice(s, u)
                    if s == 0 and u == 0:
                        # split v=0 out so it only gates on transfer T1a:
                        # the first matmul chain starts ~1.5us earlier
                        nc.vector.tensor_mul(
                            out=rhs[:, :BPC], in0=in0,
                            in1=g2t[0][:, :BPC])
                        nc.vector.tensor_mul(
                            out=rhs[:, BPC:].rearrange(
                                "p (v c) -> p v c", v=3),
                            in0=in0.unsqueeze(1).broadcast_to([128, 3, BPC]),
                            in1=g2t[0][:, BPC:].rearrange(
                                "p (v c) -> p v c", v=3))
                    else:
                        nc.vector.tensor_mul(
                            out=rhs.rearrange("p (v c) -> p v c", v=4),
                            in0=in0.unsqueeze(1).broadcast_to([128, 4, BPC]),
                            in1=g2t[s].rearrange("p (v c) -> p v c", v=4))
                    ps = psumpool.tile([128, 4 * BPC], f32, tag="ps")
                    for v in range(NMULT):
                        nc.tensor.matmul(
                            ps[:, v * BPC:(v + 1) * BPC], wt[s],
                            rhs[:, v * BPC:(v + 1) * BPC],
                            start=True, stop=True)
                    ot = sb.tile([128, 4 * BPC], bf16, tag="ot", bufs=8)
                    # vector runs all 8 products; give scalar 3 of
                    # every 4 psum drains so vector only takes u=3.
                    # The final supertile drains in parallel halves and
                    # ships on two DMA rings to shorten the tail.
                    if s == 1 and u == 3:
                        # two separate ot tiles: Tile serializes writers
                        # to one tile, so a shared tile would run the
                        # halves back-to-back instead of ACT||DVE; the
                        # second half ships on the scalar HWDGE ring
                        # (gpsimd SWDGE pays ~1us descriptor generation)
                        # DVE's half is emitted FIRST: Tile orders same-psum
                        # readers by emission, and DVE is free well before
                        # ACT finishes tile 6
                        o7b = sb.tile([128, 2 * BPC], bf16, tag="o7b",
                                      name="o7b", bufs=1)
                        nc.vector.tensor_copy(out=o7b, in_=ps[:, 2 * BPC:])
                        nc.scalar.copy(out=ot[:, :2 * BPC],
                                       in_=ps[:, :2 * BPC])
                        nc.scalar.dma_start(out=o[7, :, 2 * BPC:],
                                            in_=o7b)
                        nc.sync.dma_start(out=o[7, :, :2 * BPC],
                                          in_=ot[:, :2 * BPC])
                    elif u != 3:
                        nc.scalar.copy(out=ot, in_=ps)
                        nc.sync.dma_start(out=o[s * 4 + u, :, :], in_=ot)
                    else:
                        nc.vector.tensor_copy(out=ot, in_=ps)
                        nc.sync.dma_start(out=o[s * 4 + u, :, :], in_=ot)

    orig = nc.to_json_bytes
    nc.to_json_bytes = lambda: _dedup_ldweights(_hoist_preamble(_split_waits(orig())))
    _nc_cache = nc
    return nc


def kernel(in1, in2, cb, _want_stats=False):
    from concourse.bass_utils import run_bass_kernel_spmd

    _patch_walrus_flags()

    in1 = np.ascontiguousarray(np.asarray(in1, dtype=np.float32))
    in2 = np.ascontiguousarray(np.asarray(in2, dtype=np.float32))
    cb = np.asarray(cb, dtype=np.float32)
    groups, hglob = _build_decomp()

    wmat = np.zeros((2, 128, 128), dtype=np.float32)
    for s, g in enumerate(groups):
        wmat[s][g["w_k"], g["w_m"]] = cb[g["w_h"], g["w_c"]]
    wmat = wmat.astype(BF16)

    wcat = np.concatenate([wmat[0], wmat[1]], axis=1)
    in_maps = []
    for c in range(NCORES):
        sl = slice(c * BPC, (c + 1) * BPC)
        b1 = in1[sl].T.astype(BF16)
        b2 = in2[sl].T.astype(BF16)
        gg1 = np.empty((2, 128, 4 * BPC), dtype=BF16)
        gg2 = np.empty((2, 128, 4 * BPC), dtype=BF16)
        for s, g in enumerate(groups):
            for u in range(NMULT):
                gg1[s][:, u * BPC:(u + 1) * BPC] = b1[g["c1"][u]]
                gg2[s][:, u * BPC:(u + 1) * BPC] = b2[g["c2"][u]]
        # inA = [w0|w1 | g1_0u0 | g2_0 | g1_0u1..3]
        in_maps.append({
            "inA": np.concatenate(
                [wcat, gg1[0][:, :BPC], gg2[0], gg1[0][:, BPC:]], axis=1),
            "inC": np.concatenate([gg2[1], gg1[1]], axis=1)})

    nc = _build_nc()
    import os
    trace = bool(int(os.environ.get("KERNEL_TRACE", "0")))
    res = run_bass_kernel_spmd(nc, in_maps, core_ids=list(range(NCORES)),
                               trace=trace)

    full = np.concatenate(
        [np.asarray(r["o"], dtype=np.float32)
         .reshape(8, 128, 4, BPC).transpose(0, 2, 1, 3).reshape(32 * 128, BPC)
         for r in res.results], axis=1)
    out = np.empty((B, B), dtype=np.float32)
    out[:, hglob] = full.T
    if _want_stats:
        return out, res
    return out


if __name__ == "__main__":
    rng = np.random.default_rng(0)
    a = rng.standard_normal((B, DIM)).astype(np.float32)
    b = rng.standard_normal((B, DIM)).astype(np.float32)
    cb = np.load("/tmp/cb.npy")
    out = kernel(a, b, cb)
    outer = np.einsum("bi,bj->bij", a, b).reshape(B, -1)
    exp = outer @ cb.T
    print("rel err:", np.linalg.norm(out - exp) / np.linalg.norm(exp))

